# revision 10
# baseline (speedup 1.0000x reference)
"""Contracting-REN forward kernel for 8 Trainium2 NeuronCores.

Strategy
--------
Batch (nb=4096) is sharded 8 ways (512 rows/core); all parameters are
replicated (pure data parallelism, no collectives).

Host (inside kernel(), float64 numpy): the batch-independent "frame"
computation — H = X X^T + eps I, its blocks, E^-1, Lambda, D11 — and the
derived fp32 operand matrices laid out exactly as the device matmuls
need them (contraction dim leading, i.e. pre-transposed lhsT/rhs forms).
All parameters are packed into one [128, PK_COLS] array so the device
fetches them with a single DMA.

Device (Bass/Tile, per core): the batch-dependent math in q-major layout
  base^T = C1p^T-as-lhsT @ x^T + D12p^T @ u^T            (PE)
  W      = tanh(base^T)                                   (warm start)
  repeat NIT times:  W <- tanh(base^T + Dp-blocks @ W)    (global Picard)
  dx     = x @ EiF^T + w @ EiB1^T + u @ EiB2^T            (batch-major out)
  y^T    = C2/D21/D22 matmuls, then PE-transpose to y

The nq-step forward substitution through tanh is replaced by a global
Picard fixed-point iteration: D11 is strictly lower triangular, so the
iteration is exact after <=nq rounds and in practice contracts by ~3x
per round (measured: 12 rounds reach the fp32 noise floor, max rel err
~1e-6 on dx/y vs the reference).
"""

import functools

import numpy as np

NB, NX, NY, NU, NQ = 4096, 256, 128, 128, 256
ALPHA, EPSILON = 1.0, 1e-3
NCORES = 8
NBS = NB // NCORES  # 512 batch rows per core
NGB = NBS // 128    # 4 batch tiles of 128 per core
NXU = NX + NU       # packed x|u row width
NXY = NX + NY       # packed dx|y row width
NIT = 12            # Picard iterations after the warm start

# Packed-parameter column layouts: name -> (n rows of 128 x cols)
# "early" blob: identity + everything the recurrence needs;
# "late" blob: output-phase operands (DMA overlaps the Picard loop).
_PKE_SPECS = [
    ("ident", 128, 128), ("DpT", NQ, NQ), ("C1pT", NX, NQ),
    ("D12pT", NU, NQ), ("bvp", NQ, 1),
]
_PKL_SPECS = [
    ("EiFT", NX, NX), ("EiB1T", NQ, NX), ("EiB2T", NU, NX),
    ("C2T", NX, NY), ("D21T", NQ, NY), ("D22T", NU, NY),
]


def _pack_offsets(specs):
    off = {}
    o = 0
    for n_, r_, c_ in specs:
        off[n_] = o
        o += (r_ // 128) * c_
    return off, o


_PKE_OFF, PKE_COLS = _pack_offsets(_PKE_SPECS)
_PKL_OFF, PKL_COLS = _pack_offsets(_PKL_SPECS)

# Results of the last device run (test harness reads exec_time_ns off this).
LAST_RESULT = None


def _frame(X, Y, D12, B2, C2, D21, D22, bv):
    """Batch-independent parameter derivation (float64), packed for the DMA."""
    X = X.astype(np.float64)
    Y = Y.astype(np.float64)
    n = 2 * NX + NQ
    H = X @ X.T + EPSILON * np.eye(n)
    H11 = H[:NX, :NX]
    H21 = H[NX:NX + NQ, :NX]
    H22 = H[NX:NX + NQ, NX:NX + NQ]
    H31 = H[NX + NQ:, :NX]
    H32 = H[NX + NQ:, NX:NX + NQ]
    H33 = H[NX + NQ:, NX + NQ:]
    F, B1, P, C1 = H31, H32, H33, -H21
    E = 0.5 * (H11 + P / (ALPHA ** 2) + Y - Y.T)
    lam = 0.5 * np.diag(H22)
    D11 = -np.tril(H22, -1)
    Einv = np.linalg.inv(E)

    mats = {
        "C1pT": (C1 / lam[:, None]).T,                       # [NX, NQ]
        "D12pT": (D12.astype(np.float64) / lam[:, None]).T,  # [NU, NQ]
        "DpT": (D11 / lam[:, None]).T,                       # [NQ, NQ]
        "EiFT": (Einv @ F).T,                                # [NX, NX]
        "EiB1T": (Einv @ B1).T,                              # [NQ, NX]
        "EiB2T": (Einv @ B2.astype(np.float64)).T,           # [NU, NX]
        "C2T": C2.astype(np.float64).T,                      # [NX, NY]
        "D21T": D21.astype(np.float64).T,                    # [NQ, NY]
        "D22T": D22.astype(np.float64).T,                    # [NU, NY]
        "bvp": bv.astype(np.float64) / lam[:, None],         # [NQ, 1]
    }
    mats["ident"] = np.eye(128)

    def pack(specs, offs, cols_total):
        pk = np.zeros((128, cols_total), dtype=np.float32)
        for name, rows, cols in specs:
            a = mats[name].astype(np.float32)
            o = offs[name]
            for i in range(rows // 128):
                pk[:, o + i * cols:o + (i + 1) * cols] = a[i * 128:(i + 1) * 128]
        return pk

    return pack(_PKE_SPECS, _PKE_OFF, PKE_COLS), pack(_PKL_SPECS, _PKL_OFF, PKL_COLS)


@functools.lru_cache(maxsize=1)
def _build_program():
    import concourse.bass as bass
    from concourse import bacc
    import concourse.mybir as mybir
    import concourse.tile as tile
    from contextlib import ExitStack

    f32 = mybir.dt.float32
    f32r = mybir.dt.float32r

    def r(ap):
        # fp32 matmuls run 4 cyc/row; float32r (same bits) streams 1 cyc/row
        return ap.bitcast(f32r)

    nc = bacc.Bacc()
    xu_sh = nc.dram_tensor("xu_sh", [NBS, NXU], f32, kind="ExternalInput")
    pke_d = nc.dram_tensor("pke", [128, PKE_COLS], f32, kind="ExternalInput")
    pkl_d = nc.dram_tensor("pkl", [128, PKL_COLS], f32, kind="ExternalInput")
    dxy_sh = nc.dram_tensor("dxy_sh", [NBS, NXY], f32, kind="ExternalOutput")

    TANH = mybir.ActivationFunctionType.Tanh

    with ExitStack() as ctx:
        tc = ctx.enter_context(tile.TileContext(nc))
        cpool = ctx.enter_context(tc.tile_pool(name="const", bufs=1))
        wpool = ctx.enter_context(tc.tile_pool(name="work", bufs=2))
        wts = ctx.enter_context(tc.tile_pool(name="wts", bufs=2))
        ppool = ctx.enter_context(tc.tile_pool(name="psum", bufs=2, space="PSUM"))

        # ---- blob parameter fetches (early first; late overlaps the loop) ----
        pke_sb = cpool.tile([128, PKE_COLS], f32r)
        pkl_sb = cpool.tile([128, PKL_COLS], f32r)
        nc.sync.dma_start(out=pke_sb, in_=pke_d[:, :].bitcast(f32r))

        def par(name, i, cols):
            if name in _PKE_OFF:
                o = _PKE_OFF[name] + i * cols
                return pke_sb[:, o:o + cols]
            o = _PKL_OFF[name] + i * cols
            return pkl_sb[:, o:o + cols]

        ident = par("ident", 0, 128)

        C1pT = [par("C1pT", i, NQ) for i in range(2)]
        D12pT = par("D12pT", 0, NQ)
        DpT = [par("DpT", i, NQ) for i in range(2)]
        EiFT = [par("EiFT", i, NX) for i in range(2)]
        EiB1T = [par("EiB1T", i, NX) for i in range(2)]
        EiB2T = par("EiB2T", 0, NX)
        C2T = [par("C2T", i, NY) for i in range(2)]
        D21T = [par("D21T", i, NY) for i in range(2)]
        D22T = par("D22T", 0, NY)
        bvp = [par("bvp", i, 1) for i in range(2)]

        # ---- one-DMA x|u fetch: [512, 384] -> [128, 4, 384] ----
        xu_t = xu_sh[:, :].rearrange("(g p) m -> p g m", p=128)  # partition-major view
        xu_sb = cpool.tile([128, NGB, NXU], f32)
        nc.sync.dma_start(out=xu_sb, in_=xu_t)
        nc.sync.dma_start(out=pkl_sb, in_=pkl_d[:, :].bitcast(f32r))

        # PE matmuls accept only one semaphore wait; this throwaway
        # transpose makes the PE observe the early-blob DMA before the
        # x/u transposes (which then wait only on the x/u DMA).
        dummy = ppool.tile([128, 128], f32, tag="tp", bufs=2)
        nc.tensor.transpose(dummy, ident.bitcast(f32), ident.bitcast(f32))

        # ---- transpose x, u into q-major xT [NX, NBS], uT [NU, NBS] ----
        xT = [cpool.tile([128, NBS], f32r, name=f"xT{i}") for i in range(2)]
        uT = cpool.tile([128, NBS], f32r, name="uT")
        for bt in range(NGB):
            cs = slice(bt * 128, (bt + 1) * 128)
            for nt in range(2):
                pt = ppool.tile([128, 128], f32, tag="tp", bufs=2)
                nc.tensor.transpose(pt, xu_sb[:, bt, nt * 128:(nt + 1) * 128], ident.bitcast(f32))
                nc.vector.tensor_copy(xT[nt][:, cs], pt)
            pt = ppool.tile([128, 128], f32, tag="tp", bufs=2)
            nc.tensor.transpose(pt, xu_sb[:, bt, NX:NXU], ident.bitcast(f32))
            nc.vector.tensor_copy(uT[:, cs], pt)

        # ---- base^T (q-major) + warm start W = tanh(base^T) ----
        baseT = [cpool.tile([128, NBS], f32r, name=f"baseT{t}") for t in range(2)]
        W = []
        for t in range(2):
            ts = slice(t * 128, (t + 1) * 128)
            ps = ppool.tile([128, NBS], f32, tag=f"ps{t}", bufs=2)
            nc.tensor.matmul(ps, C1pT[0][:, ts], xT[0], start=True, stop=False)
            nc.tensor.matmul(ps, C1pT[1][:, ts], xT[1], start=False, stop=False)
            nc.tensor.matmul(ps, D12pT[:, ts], uT, start=False, stop=True)
            # base^T = psum + bv/lambda  (per-partition bias)
            nc.scalar.add(baseT[t], ps, add=bvp[t].bitcast(f32))
            w0 = wts.tile([128, NBS], f32r, tag=f"W{t}", bufs=2, name=f"W{t}_0")
            nc.scalar.activation(w0, baseT[t], TANH)
            W.append(w0)

        # ---- global Picard iterations ----
        for it in range(NIT):
            Wn = []
            ps0 = ppool.tile([128, NBS], f32, tag="ps0", bufs=2)
            nc.tensor.matmul(ps0, ident, baseT[0], start=True, stop=False)
            nc.tensor.matmul(ps0, DpT[0][:, 0:128], W[0], start=False, stop=True)
            w0 = wts.tile([128, NBS], f32r, tag="W0", bufs=2, name=f"W0_{it + 1}")
            nc.scalar.activation(w0, ps0, TANH)
            Wn.append(w0)

            ps1 = ppool.tile([128, NBS], f32, tag="ps1", bufs=2)
            nc.tensor.matmul(ps1, ident, baseT[1], start=True, stop=False)
            nc.tensor.matmul(ps1, DpT[0][:, 128:256], W[0], start=False, stop=False)
            nc.tensor.matmul(ps1, DpT[1][:, 128:256], W[1], start=False, stop=True)
            w1 = wts.tile([128, NBS], f32r, tag="W1", bufs=2, name=f"W1_{it + 1}")
            nc.scalar.activation(w1, ps1, TANH)
            Wn.append(w1)
            W = Wn

        # ---- outputs, packed [128, 4, 384] then one DMA out ----
        out_sb = cpool.tile([128, NGB, NXY], f32)

        # dx = x EiF^T + w EiB1^T + u EiB2^T  (batch-major)
        for bt in range(NGB):
            bs = slice(bt * 128, (bt + 1) * 128)
            ps = ppool.tile([128, NX], f32, tag="op", bufs=2)
            nc.tensor.matmul(ps, xT[0][:, bs], EiFT[0], start=True, stop=False)
            nc.tensor.matmul(ps, xT[1][:, bs], EiFT[1], start=False, stop=False)
            nc.tensor.matmul(ps, W[0][:, bs], EiB1T[0], start=False, stop=False)
            nc.tensor.matmul(ps, W[1][:, bs], EiB1T[1], start=False, stop=False)
            nc.tensor.matmul(ps, uT[:, bs], EiB2T, start=False, stop=True)
            nc.vector.tensor_copy(out_sb[:, bt, 0:NX], ps)

        # y^T = C2 x^T + D21 w + D22 u^T (q-major), transposed out per tile
        psy = ppool.tile([128, NBS], f32, tag="op", bufs=2)
        nc.tensor.matmul(psy, C2T[0], xT[0], start=True, stop=False)
        nc.tensor.matmul(psy, C2T[1], xT[1], start=False, stop=False)
        nc.tensor.matmul(psy, D21T[0], W[0], start=False, stop=False)
        nc.tensor.matmul(psy, D21T[1], W[1], start=False, stop=False)
        nc.tensor.matmul(psy, D22T, uT, start=False, stop=True)
        yTt = wpool.tile([128, NBS], f32, tag="yT", bufs=1)
        nc.vector.tensor_copy(yTt, psy)
        for bt in range(NGB):
            pt = ppool.tile([128, 128], f32, tag="tp", bufs=2)
            nc.tensor.transpose(pt, yTt[:, bt * 128:(bt + 1) * 128], ident.bitcast(f32))
            nc.vector.tensor_copy(out_sb[:, bt, NX:NXY], pt)

        dxy_t = dxy_sh[:, :].rearrange("(g p) m -> p g m", p=128)
        nc.sync.dma_start(out=dxy_t, in_=out_sb)

    nc.finalize()
    return nc


def kernel(x, u, X, Y, D12, B2, C2, D21, D22, bv):
    global LAST_RESULT
    import os

    x = np.asarray(x, dtype=np.float32)
    u = np.asarray(u, dtype=np.float32)
    xu = np.concatenate([x, u], axis=1)  # [NB, NXU]
    pke, pkl = _frame(
        np.asarray(X), np.asarray(Y), np.asarray(D12), np.asarray(B2),
        np.asarray(C2), np.asarray(D21), np.asarray(D22), np.asarray(bv),
    )

    nc = _build_program()
    from concourse.bass_utils import run_bass_kernel_spmd

    in_maps = [
        {"pke": pke, "pkl": pkl,
         "xu_sh": np.ascontiguousarray(xu[c * NBS:(c + 1) * NBS])}
        for c in range(NCORES)
    ]

    trace = bool(int(os.environ.get("REN_TRACE", "0")))
    res = run_bass_kernel_spmd(nc, in_maps, core_ids=list(range(NCORES)), trace=trace)
    LAST_RESULT = res

    dxy = np.concatenate([res.results[c]["dxy_sh"] for c in range(NCORES)], axis=0)
    dx = np.ascontiguousarray(dxy[:, :NX])
    y = np.ascontiguousarray(dxy[:, NX:])
    return dx, y


# revision 11
# speedup vs baseline: 1.0177x; 1.0177x over previous
"""Contracting-REN forward kernel for 8 Trainium2 NeuronCores.

Strategy
--------
Batch (nb=4096) is sharded 8 ways (512 rows/core); all parameters are
replicated (pure data parallelism, no collectives).

Host (inside kernel(), float64 numpy): the batch-independent "frame"
computation — H = X X^T + eps I, its blocks, E^-1, Lambda, D11 — and the
derived fp32 operand matrices laid out exactly as the device matmuls
need them (contraction dim leading, i.e. pre-transposed lhsT/rhs forms),
packed into two blobs (recurrence operands / output operands) so the
device fetches them with two DMAs.

Device (Bass/Tile, per core): the batch-dependent math in q-major layout
  base^T = C1p^T-as-lhsT @ x^T + D12p^T @ u^T            (PE)
  W      = tanh(base^T)                                   (warm start)
  NIT_BF16 iterations of  W <- tanh(base^T + Dp @ W)      (bf16 operands)
  NIT_F32R iterations of the same in fp32r                (polish)
  dx     = x @ EiF^T + w @ EiB1^T + u @ EiB2^T            (batch-major out)
  y^T    = C2/D21/D22 matmuls, then PE-transpose to y

The nq-step forward substitution through tanh is replaced by a global
Picard fixed-point iteration: D11 is strictly lower triangular, so the
iteration is exact after <=nq rounds and in practice contracts by ~3x
per round. Early rounds run with bf16 matmul operands (1 cyc/row on the
PE vs ~2 for fp32r); the fp32r polish rounds then converge to the fp32
fixed point. Throwaway matmuls at kernel start keep the PE's HAM clock
gate warm through the DMA phase.
"""

import functools

import numpy as np

NB, NX, NY, NU, NQ = 4096, 256, 128, 128, 256
ALPHA, EPSILON = 1.0, 1e-3
NCORES = 8
NBS = NB // NCORES  # 512 batch rows per core
NGB = NBS // 128    # 4 batch tiles of 128 per core
NXU = NX + NU       # packed x|u row width
NXY = NX + NY       # packed dx|y row width
NIT_BF16 = 7        # bf16 Picard iterations after the warm start
NIT_F32R = 4        # fp32r polish iterations
N_WARMUP = 9        # throwaway PE matmuls to lift the HAM clock gate

# Packed-parameter column layouts: name -> (n rows of 128, cols).
# "early" blob: everything the recurrence needs; "late" blob: output-phase
# operands (its DMA overlaps the Picard loop).
_PKE_SPECS = [
    ("DpT", NQ, NQ), ("C1pT", NX, NQ), ("D12pT", NU, NQ), ("bvp", NQ, 1),
]
_PKL_SPECS = [
    ("EiFT", NX, NX), ("EiB1T", NQ, NX), ("EiB2T", NU, NX),
    ("C2T", NX, NY), ("D21T", NQ, NY), ("D22T", NU, NY),
]


def _pack_offsets(specs):
    off = {}
    o = 0
    for n_, r_, c_ in specs:
        off[n_] = o
        o += (r_ // 128) * c_
    return off, o


_PKE_OFF, PKE_COLS = _pack_offsets(_PKE_SPECS)
_PKL_OFF, PKL_COLS = _pack_offsets(_PKL_SPECS)

# Results of the last device run (test harness reads exec_time_ns off this).
LAST_RESULT = None


def _frame(X, Y, D12, B2, C2, D21, D22, bv):
    """Batch-independent parameter derivation (float64), packed for the DMA."""
    X = X.astype(np.float64)
    Y = Y.astype(np.float64)
    n = 2 * NX + NQ
    H = X @ X.T + EPSILON * np.eye(n)
    H11 = H[:NX, :NX]
    H21 = H[NX:NX + NQ, :NX]
    H22 = H[NX:NX + NQ, NX:NX + NQ]
    H31 = H[NX + NQ:, :NX]
    H32 = H[NX + NQ:, NX:NX + NQ]
    H33 = H[NX + NQ:, NX + NQ:]
    F, B1, P, C1 = H31, H32, H33, -H21
    E = 0.5 * (H11 + P / (ALPHA ** 2) + Y - Y.T)
    lam = 0.5 * np.diag(H22)
    D11 = -np.tril(H22, -1)
    Einv = np.linalg.inv(E)

    mats = {
        "C1pT": (C1 / lam[:, None]).T,                       # [NX, NQ]
        "D12pT": (D12.astype(np.float64) / lam[:, None]).T,  # [NU, NQ]
        "DpT": (D11 / lam[:, None]).T,                       # [NQ, NQ]
        "EiFT": (Einv @ F).T,                                # [NX, NX]
        "EiB1T": (Einv @ B1).T,                              # [NQ, NX]
        "EiB2T": (Einv @ B2.astype(np.float64)).T,           # [NU, NX]
        "C2T": C2.astype(np.float64).T,                      # [NX, NY]
        "D21T": D21.astype(np.float64).T,                    # [NQ, NY]
        "D22T": D22.astype(np.float64).T,                    # [NU, NY]
        "bvp": bv.astype(np.float64) / lam[:, None],         # [NQ, 1]
    }

    def pack(specs, offs, cols_total):
        pk = np.zeros((128, cols_total), dtype=np.float32)
        for name, rows, cols in specs:
            a = mats[name].astype(np.float32)
            o = offs[name]
            for i in range(rows // 128):
                pk[:, o + i * cols:o + (i + 1) * cols] = a[i * 128:(i + 1) * 128]
        return pk

    return pack(_PKE_SPECS, _PKE_OFF, PKE_COLS), pack(_PKL_SPECS, _PKL_OFF, PKL_COLS)


@functools.lru_cache(maxsize=1)
def _build_program():
    import concourse.bass as bass
    from concourse import bacc
    import concourse.mybir as mybir
    import concourse.tile as tile
    from concourse.masks import make_identity
    from contextlib import ExitStack

    f32 = mybir.dt.float32
    f32r = mybir.dt.float32r
    bf16 = mybir.dt.bfloat16
    TANH = mybir.ActivationFunctionType.Tanh

    nc = bacc.Bacc()
    xu_sh = nc.dram_tensor("xu_sh", [NBS, NXU], f32, kind="ExternalInput")
    pke_d = nc.dram_tensor("pke", [128, PKE_COLS], f32, kind="ExternalInput")
    pkl_d = nc.dram_tensor("pkl", [128, PKL_COLS], f32, kind="ExternalInput")
    dxy_sh = nc.dram_tensor("dxy_sh", [NBS, NXY], f32, kind="ExternalOutput")

    with ExitStack() as ctx:
        tc = ctx.enter_context(tile.TileContext(nc))
        cpool = ctx.enter_context(tc.tile_pool(name="const", bufs=1))
        wpool = ctx.enter_context(tc.tile_pool(name="work", bufs=2))
        wts = ctx.enter_context(tc.tile_pool(name="wts", bufs=2))
        ppool = ctx.enter_context(tc.tile_pool(name="psum", bufs=2, space="PSUM"))

        # ---- PE warm-up + tanh table preload, overlapping the input DMAs.
        # HAM releases the PE clock gate (1.2 -> 2.4 GHz) only after ~3.4us
        # of sustained matmul activity; burn that in on zeros now.
        warm = cpool.tile([128, 512], bf16, name="warm")
        nc.vector.memset(warm, 0.0)
        scr = cpool.tile([128, 1], f32, name="scr")
        nc.scalar.activation(scr, warm[:, 0:1], TANH)  # pulls ACT_TABLE_LOAD early
        for i in range(N_WARMUP):
            wps = ppool.tile([128, 512], f32, tag="op", bufs=1, name=f"wps{i}")
            nc.tensor.matmul(wps, warm[:, 0:128], warm, start=True, stop=True)

        # ---- input DMAs: x|u first (transposes depend on it), then the
        # recurrence blob; the output blob rides a different queue.
        xu_t = xu_sh[:, :].rearrange("(g p) m -> p g m", p=128)
        xu_sb = cpool.tile([128, NGB, NXU], f32)
        nc.sync.dma_start(out=xu_sb, in_=xu_t)
        pke_sb = cpool.tile([128, PKE_COLS], f32r)
        nc.sync.dma_start(out=pke_sb, in_=pke_d[:, :].bitcast(f32r))
        pkl_sb = cpool.tile([128, PKL_COLS], f32r)
        nc.scalar.dma_start(out=pkl_sb, in_=pkl_d[:, :].bitcast(f32r))

        def par(name, i, cols):
            if name in _PKE_OFF:
                o = _PKE_OFF[name] + i * cols
                return pke_sb[:, o:o + cols]
            o = _PKL_OFF[name] + i * cols
            return pkl_sb[:, o:o + cols]

        C1pT = [par("C1pT", i, NQ) for i in range(2)]
        D12pT = par("D12pT", 0, NQ)
        DpT = [par("DpT", i, NQ) for i in range(2)]
        bvp = [par("bvp", i, 1).bitcast(f32) for i in range(2)]
        EiFT = [par("EiFT", i, NX) for i in range(2)]
        EiB1T = [par("EiB1T", i, NX) for i in range(2)]
        EiB2T = par("EiB2T", 0, NX)
        C2T = [par("C2T", i, NY) for i in range(2)]
        D21T = [par("D21T", i, NY) for i in range(2)]
        D22T = par("D22T", 0, NY)

        ident = cpool.tile([128, 128], f32)
        make_identity(nc, ident)

        # bf16 copies of the loop operands
        identb = cpool.tile([128, 128], bf16, name="identb")
        nc.vector.tensor_copy(identb, ident)
        DpTb = []
        for i in range(2):
            t = cpool.tile([128, NQ], bf16, name=f"DpTb{i}")
            nc.vector.tensor_copy(t, DpT[i])
            DpTb.append(t)

        # ---- transpose x, u into q-major xT [NX, NBS], uT [NU, NBS] ----
        xT = [cpool.tile([128, NBS], f32r, name=f"xT{i}") for i in range(2)]
        uT = cpool.tile([128, NBS], f32r, name="uT")
        for bt in range(NGB):
            cs = slice(bt * 128, (bt + 1) * 128)
            for nt in range(2):
                pt = ppool.tile([128, 128], f32, tag="tp", bufs=2)
                nc.tensor.transpose(pt, xu_sb[:, bt, nt * 128:(nt + 1) * 128], ident)
                nc.vector.tensor_copy(xT[nt][:, cs], pt)
            pt = ppool.tile([128, 128], f32, tag="tp", bufs=2)
            nc.tensor.transpose(pt, xu_sb[:, bt, NX:NXU], ident)
            nc.vector.tensor_copy(uT[:, cs], pt)

        # ---- base^T (q-major), warm start W = tanh(base^T + bv') ----
        baseT = [cpool.tile([128, NBS], f32r, name=f"baseT{t}") for t in range(2)]
        baseTb = [cpool.tile([128, NBS], bf16, name=f"baseTb{t}") for t in range(2)]
        Wb = []
        for t in range(2):
            ts = slice(t * 128, (t + 1) * 128)
            ps = ppool.tile([128, NBS], f32, tag=f"ps{t}", bufs=2)
            nc.tensor.matmul(ps, C1pT[0][:, ts], xT[0], start=True, stop=False)
            nc.tensor.matmul(ps, C1pT[1][:, ts], xT[1], start=False, stop=False)
            nc.tensor.matmul(ps, D12pT[:, ts], uT, start=False, stop=True)
            nc.vector.tensor_scalar_add(baseT[t], ps, bvp[t])
            w0 = wts.tile([128, NBS], bf16, tag=f"Wb{t}", bufs=2, name=f"Wb{t}_0")
            nc.scalar.activation(w0, ps, TANH, bias=bvp[t])
            nc.vector.tensor_copy(baseTb[t], baseT[t])
            Wb.append(w0)

        # ---- bf16 Picard iterations ----
        for it in range(NIT_BF16):
            Wn = []
            ps0 = ppool.tile([128, NBS], f32, tag="ps0", bufs=2)
            nc.tensor.matmul(ps0, identb, baseTb[0], start=True, stop=False)
            nc.tensor.matmul(ps0, DpTb[0][:, 0:128], Wb[0], start=False, stop=True)
            w0 = wts.tile([128, NBS], bf16, tag="Wb0", bufs=2, name=f"Wb0_{it + 1}")
            nc.scalar.activation(w0, ps0, TANH)
            Wn.append(w0)

            ps1 = ppool.tile([128, NBS], f32, tag="ps1", bufs=2)
            nc.tensor.matmul(ps1, identb, baseTb[1], start=True, stop=False)
            nc.tensor.matmul(ps1, DpTb[0][:, 128:256], Wb[0], start=False, stop=False)
            nc.tensor.matmul(ps1, DpTb[1][:, 128:256], Wb[1], start=False, stop=True)
            w1 = wts.tile([128, NBS], bf16, tag="Wb1", bufs=2, name=f"Wb1_{it + 1}")
            nc.scalar.activation(w1, ps1, TANH)
            Wn.append(w1)
            Wb = Wn

        # ---- transition to fp32r ----
        W = []
        for t in range(2):
            w = wts.tile([128, NBS], f32r, tag=f"W{t}", bufs=2, name=f"W{t}_c")
            nc.vector.tensor_copy(w, Wb[t])
            W.append(w)

        # ---- fp32r polish iterations (base added on the DVE) ----
        for it in range(NIT_F32R):
            Wn = []
            ps0 = ppool.tile([128, NBS], f32, tag="ps0", bufs=2)
            nc.tensor.matmul(ps0, DpT[0][:, 0:128], W[0], start=True, stop=True)
            tmp0 = wpool.tile([128, NBS], f32, tag="tmp0", bufs=2)
            nc.vector.tensor_add(tmp0, ps0, baseT[0].bitcast(f32))
            w0 = wts.tile([128, NBS], f32r, tag="W0", bufs=2, name=f"W0_{it + 1}")
            nc.scalar.activation(w0, tmp0, TANH)
            Wn.append(w0)

            ps1 = ppool.tile([128, NBS], f32, tag="ps1", bufs=2)
            nc.tensor.matmul(ps1, DpT[0][:, 128:256], W[0], start=True, stop=False)
            nc.tensor.matmul(ps1, DpT[1][:, 128:256], W[1], start=False, stop=True)
            tmp1 = wpool.tile([128, NBS], f32, tag="tmp1", bufs=2)
            nc.vector.tensor_add(tmp1, ps1, baseT[1].bitcast(f32))
            w1 = wts.tile([128, NBS], f32r, tag="W1", bufs=2, name=f"W1_{it + 1}")
            nc.scalar.activation(w1, tmp1, TANH)
            Wn.append(w1)
            W = Wn

        # ---- outputs: y^T first (q-major), then per-batch-tile dx plus the
        # transposed y slice, each tile DMA'd out as soon as it's ready.
        out_sb = cpool.tile([128, NGB, NXY], f32)
        dxy_t = dxy_sh[:, :].rearrange("(g p) m -> p g m", p=128)

        psy = ppool.tile([128, NBS], f32, tag="op", bufs=1)
        nc.tensor.matmul(psy, C2T[0], xT[0], start=True, stop=False)
        nc.tensor.matmul(psy, C2T[1], xT[1], start=False, stop=False)
        nc.tensor.matmul(psy, D21T[0], W[0], start=False, stop=False)
        nc.tensor.matmul(psy, D21T[1], W[1], start=False, stop=False)
        nc.tensor.matmul(psy, D22T, uT, start=False, stop=True)
        yTt = wpool.tile([128, NBS], f32, tag="yT", bufs=1)
        nc.vector.tensor_copy(yTt, psy)

        for bt in range(NGB):
            bs = slice(bt * 128, (bt + 1) * 128)
            ps = ppool.tile([128, NX], f32, tag="ps0" if bt % 2 == 0 else "ps1", bufs=2)
            nc.tensor.matmul(ps, xT[0][:, bs], EiFT[0], start=True, stop=False)
            nc.tensor.matmul(ps, xT[1][:, bs], EiFT[1], start=False, stop=False)
            nc.tensor.matmul(ps, W[0][:, bs], EiB1T[0], start=False, stop=False)
            nc.tensor.matmul(ps, W[1][:, bs], EiB1T[1], start=False, stop=False)
            nc.tensor.matmul(ps, uT[:, bs], EiB2T, start=False, stop=True)
            nc.vector.tensor_copy(out_sb[:, bt, 0:NX], ps)
            pt = ppool.tile([128, 128], f32, tag="tp", bufs=2)
            nc.tensor.transpose(pt, yTt[:, bs], ident)
            nc.vector.tensor_copy(out_sb[:, bt, NX:NXY], pt)
            nc.sync.dma_start(out=dxy_t[:, bt, :], in_=out_sb[:, bt, :])

    nc.finalize()
    return nc


def kernel(x, u, X, Y, D12, B2, C2, D21, D22, bv):
    global LAST_RESULT
    import os

    x = np.asarray(x, dtype=np.float32)
    u = np.asarray(u, dtype=np.float32)
    xu = np.concatenate([x, u], axis=1)  # [NB, NXU]
    pke, pkl = _frame(
        np.asarray(X), np.asarray(Y), np.asarray(D12), np.asarray(B2),
        np.asarray(C2), np.asarray(D21), np.asarray(D22), np.asarray(bv),
    )

    nc = _build_program()
    from concourse.bass_utils import run_bass_kernel_spmd

    in_maps = [
        {"pke": pke, "pkl": pkl,
         "xu_sh": np.ascontiguousarray(xu[c * NBS:(c + 1) * NBS])}
        for c in range(NCORES)
    ]

    trace = bool(int(os.environ.get("REN_TRACE", "0")))
    res = run_bass_kernel_spmd(nc, in_maps, core_ids=list(range(NCORES)), trace=trace)
    LAST_RESULT = res

    dxy = np.concatenate([res.results[c]["dxy_sh"] for c in range(NCORES)], axis=0)
    dx = np.ascontiguousarray(dxy[:, :NX])
    y = np.ascontiguousarray(dxy[:, NX:])
    return dx, y


# revision 12
# speedup vs baseline: 1.1700x; 1.1496x over previous
"""Contracting-REN forward kernel for 8 Trainium2 NeuronCores.

Strategy
--------
Batch (nb=4096) is sharded 8 ways (512 rows/core); all parameters are
replicated (pure data parallelism, no collectives).

Host (inside kernel(), float64 numpy): the batch-independent "frame"
computation — H = X X^T + eps I, its blocks, E^-1, Lambda, D11 — and the
derived fp32 operand matrices laid out exactly as the device matmuls
need them (contraction dim leading, i.e. pre-transposed lhsT/rhs forms),
packed into two blobs (recurrence operands / output operands) so the
device fetches them with two DMAs.

Device (Bass/Tile, per core): the batch-dependent math in q-major layout
  base^T = C1p^T-as-lhsT @ x^T + D12p^T @ u^T            (PE)
  W      = tanh(base^T)                                   (warm start)
  NIT_BF16 iterations of  W <- tanh(base^T + Dp @ W)      (bf16 operands)
  NIT_F32R iterations of the same in fp32r                (polish)
  dx     = x @ EiF^T + w @ EiB1^T + u @ EiB2^T            (batch-major out)
  y^T    = C2/D21/D22 matmuls, then PE-transpose to y

The nq-step forward substitution through tanh is replaced by a global
Picard fixed-point iteration: D11 is strictly lower triangular, so the
iteration is exact after <=nq rounds and in practice contracts by ~3x
per round. Early rounds run with bf16 matmul operands (1 cyc/row on the
PE vs ~2 for fp32r); the fp32r polish rounds then converge to the fp32
fixed point. Throwaway matmuls at kernel start keep the PE's HAM clock
gate warm through the DMA phase.
"""

import functools

import numpy as np

NB, NX, NY, NU, NQ = 4096, 256, 128, 128, 256
ALPHA, EPSILON = 1.0, 1e-3
NCORES = 8
NBS = NB // NCORES  # 512 batch rows per core
NGB = NBS // 128    # 4 batch tiles of 128 per core
NXU = NX + NU       # packed x|u row width
NXY = NX + NY       # packed dx|y row width
NIT_BF16 = 6        # bf16 Picard iterations after the warm start
NIT_F32R = 4        # fp32r polish iterations
N_WARMUP = 10       # throwaway PE matmuls to lift the HAM clock gate

# Packed-parameter column layouts: name -> (n rows of 128, cols).
# "early" blob: everything the recurrence needs; "late" blob: output-phase
# operands (its DMA overlaps the Picard loop).
_PKE_SPECS = [
    ("DpT", NQ, NQ), ("C1pT", NX, NQ), ("D12pT", NU, NQ), ("bvp", NQ, 1),
]
_PKL_SPECS = [
    ("EiFT", NX, NX), ("EiB1T", NQ, NX), ("EiB2T", NU, NX),
    ("C2T", NX, NY), ("D21T", NQ, NY), ("D22T", NU, NY),
]


def _pack_offsets(specs):
    off = {}
    o = 0
    for n_, r_, c_ in specs:
        off[n_] = o
        o += (r_ // 128) * c_
    return off, o


_PKE_OFF, PKE_COLS = _pack_offsets(_PKE_SPECS)
_PKL_OFF, PKL_COLS = _pack_offsets(_PKL_SPECS)

# Results of the last device run (test harness reads exec_time_ns off this).
LAST_RESULT = None


def _frame(X, Y, D12, B2, C2, D21, D22, bv):
    """Batch-independent parameter derivation (float64), packed for the DMA."""
    X = X.astype(np.float64)
    Y = Y.astype(np.float64)
    n = 2 * NX + NQ
    H = X @ X.T + EPSILON * np.eye(n)
    H11 = H[:NX, :NX]
    H21 = H[NX:NX + NQ, :NX]
    H22 = H[NX:NX + NQ, NX:NX + NQ]
    H31 = H[NX + NQ:, :NX]
    H32 = H[NX + NQ:, NX:NX + NQ]
    H33 = H[NX + NQ:, NX + NQ:]
    F, B1, P, C1 = H31, H32, H33, -H21
    E = 0.5 * (H11 + P / (ALPHA ** 2) + Y - Y.T)
    lam = 0.5 * np.diag(H22)
    D11 = -np.tril(H22, -1)
    Einv = np.linalg.inv(E)

    mats = {
        "C1pT": (C1 / lam[:, None]).T,                       # [NX, NQ]
        "D12pT": (D12.astype(np.float64) / lam[:, None]).T,  # [NU, NQ]
        "DpT": (D11 / lam[:, None]).T,                       # [NQ, NQ]
        "EiFT": (Einv @ F).T,                                # [NX, NX]
        "EiB1T": (Einv @ B1).T,                              # [NQ, NX]
        "EiB2T": (Einv @ B2.astype(np.float64)).T,           # [NU, NX]
        "C2T": C2.astype(np.float64).T,                      # [NX, NY]
        "D21T": D21.astype(np.float64).T,                    # [NQ, NY]
        "D22T": D22.astype(np.float64).T,                    # [NU, NY]
        "bvp": bv.astype(np.float64) / lam[:, None],         # [NQ, 1]
    }

    def pack(specs, offs, cols_total):
        pk = np.zeros((128, cols_total), dtype=np.float32)
        for name, rows, cols in specs:
            a = mats[name].astype(np.float32)
            o = offs[name]
            for i in range(rows // 128):
                pk[:, o + i * cols:o + (i + 1) * cols] = a[i * 128:(i + 1) * 128]
        return pk

    return pack(_PKE_SPECS, _PKE_OFF, PKE_COLS), pack(_PKL_SPECS, _PKL_OFF, PKL_COLS)


@functools.lru_cache(maxsize=1)
def _build_program():
    import concourse.bass as bass
    from concourse import bacc
    import concourse.mybir as mybir
    import concourse.tile as tile
    from concourse.masks import make_identity
    from contextlib import ExitStack

    f32 = mybir.dt.float32
    f32r = mybir.dt.float32r
    bf16 = mybir.dt.bfloat16
    TANH = mybir.ActivationFunctionType.Tanh

    nc = bacc.Bacc()
    xu_sh = nc.dram_tensor("xu_sh", [NBS, NXU], f32, kind="ExternalInput")
    pke_d = nc.dram_tensor("pke", [128, PKE_COLS], f32, kind="ExternalInput")
    pkl_d = nc.dram_tensor("pkl", [128, PKL_COLS], f32, kind="ExternalInput")
    dxy_sh = nc.dram_tensor("dxy_sh", [NBS, NXY], f32, kind="ExternalOutput")

    with ExitStack() as ctx:
        tc = ctx.enter_context(tile.TileContext(nc))
        cpool = ctx.enter_context(tc.tile_pool(name="const", bufs=1))
        wpool = ctx.enter_context(tc.tile_pool(name="work", bufs=2))
        wts = ctx.enter_context(tc.tile_pool(name="wts", bufs=2))
        ppool = ctx.enter_context(tc.tile_pool(name="psum", bufs=2, space="PSUM"))

        # ---- PE warm-up + tanh table preload, overlapping the input DMAs.
        # HAM releases the PE clock gate (1.2 -> 2.4 GHz) only after ~3.4us
        # of sustained matmul activity; burn that in on zeros now.
        warm = cpool.tile([128, 512], bf16, name="warm")
        nc.vector.memset(warm, 0.0)
        scr = cpool.tile([128, 1], f32, name="scr")
        nc.scalar.activation(scr, warm[:, 0:1], TANH)  # pulls ACT_TABLE_LOAD early
        for i in range(N_WARMUP):
            wps = ppool.tile([128, 512], f32, tag="op", bufs=1, name=f"wps{i}")
            nc.tensor.matmul(wps, warm[:, 0:128], warm, start=True, stop=True)

        # ---- input DMAs: x|u first (transposes depend on it), then the
        # recurrence blob; the output blob rides a different queue.
        xu_t = xu_sh[:, :].rearrange("(g p) m -> p g m", p=128)
        xu_sb = cpool.tile([128, NGB, NXU], f32)
        nc.sync.dma_start(out=xu_sb, in_=xu_t)
        pke_sb = cpool.tile([128, PKE_COLS], f32r)
        nc.sync.dma_start(out=pke_sb, in_=pke_d[:, :].bitcast(f32r))
        pkl_sb = cpool.tile([128, PKL_COLS], f32r)
        nc.scalar.dma_start(out=pkl_sb, in_=pkl_d[:, :].bitcast(f32r))

        def par(name, i, cols):
            if name in _PKE_OFF:
                o = _PKE_OFF[name] + i * cols
                return pke_sb[:, o:o + cols]
            o = _PKL_OFF[name] + i * cols
            return pkl_sb[:, o:o + cols]

        C1pT = [par("C1pT", i, NQ) for i in range(2)]
        D12pT = par("D12pT", 0, NQ)
        DpT = [par("DpT", i, NQ) for i in range(2)]
        bvp = [par("bvp", i, 1).bitcast(f32) for i in range(2)]
        EiFT = [par("EiFT", i, NX) for i in range(2)]
        EiB1T = [par("EiB1T", i, NX) for i in range(2)]
        EiB2T = par("EiB2T", 0, NX)
        C2T = [par("C2T", i, NY) for i in range(2)]
        D21T = [par("D21T", i, NY) for i in range(2)]
        D22T = par("D22T", 0, NY)

        ident = cpool.tile([128, 128], f32)
        make_identity(nc, ident)

        # bf16 copies of the loop operands
        identb = cpool.tile([128, 128], bf16, name="identb")
        nc.vector.tensor_copy(identb, ident)
        identr = cpool.tile([128, 128], f32r, name="identr")
        nc.vector.tensor_copy(identr, ident)
        DpTb = []
        for i in range(2):
            t = cpool.tile([128, NQ], bf16, name=f"DpTb{i}")
            nc.vector.tensor_copy(t, DpT[i])
            DpTb.append(t)

        # ---- transpose x, u into q-major xT [NX, NBS], uT [NU, NBS] ----
        xT = [cpool.tile([128, NBS], f32r, name=f"xT{i}") for i in range(2)]
        uT = cpool.tile([128, NBS], f32r, name="uT")
        for bt in range(NGB):
            cs = slice(bt * 128, (bt + 1) * 128)
            for nt in range(2):
                pt = ppool.tile([128, 128], f32, tag="tp", bufs=2)
                nc.tensor.transpose(pt, xu_sb[:, bt, nt * 128:(nt + 1) * 128], ident)
                nc.vector.tensor_copy(xT[nt][:, cs], pt)
            pt = ppool.tile([128, 128], f32, tag="tp", bufs=2)
            nc.tensor.transpose(pt, xu_sb[:, bt, NX:NXU], ident)
            nc.vector.tensor_copy(uT[:, cs], pt)

        # ---- base^T (q-major), warm start W = tanh(base^T + bv') ----
        baseT = [cpool.tile([128, NBS], f32r, name=f"baseT{t}") for t in range(2)]
        baseTb = [cpool.tile([128, NBS], bf16, name=f"baseTb{t}") for t in range(2)]
        Wb = []
        for t in range(2):
            ts = slice(t * 128, (t + 1) * 128)
            ps = ppool.tile([128, NBS], f32, tag=f"ps{t}", bufs=2)
            nc.tensor.matmul(ps, C1pT[0][:, ts], xT[0], start=True, stop=False)
            nc.tensor.matmul(ps, C1pT[1][:, ts], xT[1], start=False, stop=False)
            nc.tensor.matmul(ps, D12pT[:, ts], uT, start=False, stop=True)
            nc.vector.tensor_scalar_add(baseT[t], ps, bvp[t])
            w0 = wts.tile([128, NBS], bf16, tag=f"Wb{t}", bufs=2, name=f"Wb{t}_0")
            nc.scalar.activation(w0, ps, TANH, bias=bvp[t])
            nc.vector.tensor_copy(baseTb[t], baseT[t])
            Wb.append(w0)

        # ---- bf16 Picard iterations ----
        for it in range(NIT_BF16):
            Wn = []
            ps0 = ppool.tile([128, NBS], f32, tag="ps0", bufs=2)
            nc.tensor.matmul(ps0, identb, baseTb[0], start=True, stop=False)
            nc.tensor.matmul(ps0, DpTb[0][:, 0:128], Wb[0], start=False, stop=True)
            w0 = wts.tile([128, NBS], bf16, tag="Wb0", bufs=2, name=f"Wb0_{it + 1}")
            nc.scalar.activation(w0, ps0, TANH)
            Wn.append(w0)

            ps1 = ppool.tile([128, NBS], f32, tag="ps1", bufs=2)
            nc.tensor.matmul(ps1, identb, baseTb[1], start=True, stop=False)
            nc.tensor.matmul(ps1, DpTb[0][:, 128:256], Wb[0], start=False, stop=False)
            nc.tensor.matmul(ps1, DpTb[1][:, 128:256], Wb[1], start=False, stop=True)
            w1 = wts.tile([128, NBS], bf16, tag="Wb1", bufs=2, name=f"Wb1_{it + 1}")
            nc.scalar.activation(w1, ps1, TANH)
            Wn.append(w1)
            Wb = Wn

        # ---- transition to fp32r ----
        W = []
        for t in range(2):
            w = wts.tile([128, NBS], f32r, tag=f"W{t}", bufs=2, name=f"W{t}_c")
            nc.vector.tensor_copy(w, Wb[t])
            W.append(w)

        # ---- fp32r polish iterations ----
        for it in range(NIT_F32R):
            Wn = []
            ps0 = ppool.tile([128, NBS], f32, tag="ps0", bufs=2)
            nc.tensor.matmul(ps0, identr, baseT[0], start=True, stop=False)
            nc.tensor.matmul(ps0, DpT[0][:, 0:128], W[0], start=False, stop=True)
            w0 = wts.tile([128, NBS], f32r, tag="W0", bufs=2, name=f"W0_{it + 1}")
            nc.scalar.activation(w0, ps0, TANH)
            Wn.append(w0)

            ps1 = ppool.tile([128, NBS], f32, tag="ps1", bufs=2)
            nc.tensor.matmul(ps1, identr, baseT[1], start=True, stop=False)
            nc.tensor.matmul(ps1, DpT[0][:, 128:256], W[0], start=False, stop=False)
            nc.tensor.matmul(ps1, DpT[1][:, 128:256], W[1], start=False, stop=True)
            w1 = wts.tile([128, NBS], f32r, tag="W1", bufs=2, name=f"W1_{it + 1}")
            nc.scalar.activation(w1, ps1, TANH)
            Wn.append(w1)
            W = Wn

        # ---- outputs: y^T first (q-major), then per-batch-tile dx plus the
        # transposed y slice, each tile DMA'd out as soon as it's ready.
        out_sb = cpool.tile([128, NGB, NXY], f32)
        dxy_t = dxy_sh[:, :].rearrange("(g p) m -> p g m", p=128)

        psy = ppool.tile([128, NBS], f32, tag="op", bufs=1)
        nc.tensor.matmul(psy, C2T[0], xT[0], start=True, stop=False)
        nc.tensor.matmul(psy, C2T[1], xT[1], start=False, stop=False)
        nc.tensor.matmul(psy, D21T[0], W[0], start=False, stop=False)
        nc.tensor.matmul(psy, D21T[1], W[1], start=False, stop=False)
        nc.tensor.matmul(psy, D22T, uT, start=False, stop=True)
        yTt = wpool.tile([128, NBS], f32, tag="yT", bufs=1)
        nc.vector.tensor_copy(yTt, psy)

        for bt in range(NGB):
            bs = slice(bt * 128, (bt + 1) * 128)
            ps = ppool.tile([128, NX], f32, tag="ps0" if bt % 2 == 0 else "ps1", bufs=2)
            nc.tensor.matmul(ps, xT[0][:, bs], EiFT[0], start=True, stop=False)
            nc.tensor.matmul(ps, xT[1][:, bs], EiFT[1], start=False, stop=False)
            nc.tensor.matmul(ps, W[0][:, bs], EiB1T[0], start=False, stop=False)
            nc.tensor.matmul(ps, W[1][:, bs], EiB1T[1], start=False, stop=False)
            nc.tensor.matmul(ps, uT[:, bs], EiB2T, start=False, stop=True)
            nc.vector.tensor_copy(out_sb[:, bt, 0:NX], ps)
            pt = ppool.tile([128, 128], f32, tag="tp", bufs=2)
            nc.tensor.transpose(pt, yTt[:, bs], ident)
            nc.vector.tensor_copy(out_sb[:, bt, NX:NXY], pt)
            nc.sync.dma_start(out=dxy_t[:, bt, :], in_=out_sb[:, bt, :])

    nc.finalize()
    return nc


def kernel(x, u, X, Y, D12, B2, C2, D21, D22, bv):
    global LAST_RESULT
    import os

    x = np.asarray(x, dtype=np.float32)
    u = np.asarray(u, dtype=np.float32)
    xu = np.concatenate([x, u], axis=1)  # [NB, NXU]
    pke, pkl = _frame(
        np.asarray(X), np.asarray(Y), np.asarray(D12), np.asarray(B2),
        np.asarray(C2), np.asarray(D21), np.asarray(D22), np.asarray(bv),
    )

    nc = _build_program()
    from concourse.bass_utils import run_bass_kernel_spmd

    in_maps = [
        {"pke": pke, "pkl": pkl,
         "xu_sh": np.ascontiguousarray(xu[c * NBS:(c + 1) * NBS])}
        for c in range(NCORES)
    ]

    trace = bool(int(os.environ.get("REN_TRACE", "0")))
    res = run_bass_kernel_spmd(nc, in_maps, core_ids=list(range(NCORES)), trace=trace)
    LAST_RESULT = res

    dxy = np.concatenate([res.results[c]["dxy_sh"] for c in range(NCORES)], axis=0)
    dx = np.ascontiguousarray(dxy[:, :NX])
    y = np.ascontiguousarray(dxy[:, NX:])
    return dx, y
